# revision 18
# baseline (speedup 1.0000x reference)
"""Trainium2 Bass kernel for nn_CandidateSelector (nms_detection).

Strategy (8 NeuronCores, data-parallel over the object axis):
  - Host marshals inputs: normalizes the 512 query tokens, transposes both
    operands to K-major layout (the PE contracts over the partition axis),
    rounds to the fp32r (11-bit mantissa) grid, and shards the 65536-object
    axis across 8 cores (8192 objects each).
  - Each core streams its 8 MB object shard once from HBM and computes, on
    device: the cosine-score GEMM  raw[obj, q] = objT.T @ qnT  (fp32r
    matmuls at full PE rate, fp32 PSUM accumulate), the per-object max over
    the 512 queries (DVE reduce_max over 3-bank PSUM tiles), and the
    per-object sum of squares (ACT square + ones-vector column-sum matmul
    on the PE).
  - Host gathers per-core results, forms object_scores = rowmax / ||obj||,
    and mirrors the reference's tiny sequential tail exactly in numpy:
    per-frame segment max, query-frame backward suppression, greedy
    inter-frame NMS, top-k and top-p filtering. The query argmax is
    recomputed on host only for the handful of finally-selected rows.
"""

import numpy as np

N_OBJ = 65536
D = 256
NQ = 512
N_CORES = 8
OPC = N_OBJ // N_CORES          # 8192 objects per core
# Object chunks per DMA: 20 chunks of 384 plus one of 512. A 384-object
# chunk is three 128-object blocks whose cosine tiles share one 3-bank PSUM
# tile, so each DVE reduce covers 1536 elements (amortizes the ~120-cycle
# PSUM access latency that dominates smaller reduces).
CHUNKS = [384] * 20 + [512]
MAXCHUNK = 512
BLK = 128                       # objects per output-partition block
RB = 3                          # blocks per PSUM tile / DVE reduce
N_BLK = OPC // BLK              # 64

_nc_cache = {}
last_exec_time_ns = None


def _round_fp32r(x):
    """Round fp32 values to the fp32r grid (11-bit mantissa, round-nearest).

    The PE's fp32r datapath computes exactly on values already on this grid;
    the BIR verifier requires matmul inputs to be produced pre-rounded.
    """
    b = np.ascontiguousarray(x).view(np.uint32)
    r = ((b.astype(np.uint64) + 0x800) & 0xFFFFF000).astype(np.uint32)
    return r.view(np.float32)


def _build_nc():
    import concourse.mybir as mybir
    import concourse.tile as tile
    from concourse import bacc

    f32, f32r = mybir.dt.float32, mybir.dt.float32r

    nc = bacc.Bacc("TRN2", target_bir_lowering=False, debug=False)
    objT_d = nc.dram_tensor("objT", [D, OPC], f32r, kind="ExternalInput")
    qnT_d = nc.dram_tensor("qnT", [D, NQ], f32r, kind="ExternalInput")
    ones_d = nc.dram_tensor("ones", [128, 128], f32r, kind="ExternalInput")
    rm_d = nc.dram_tensor("rowmax", [128, N_BLK], f32, kind="ExternalOutput")
    ss_d = nc.dram_tensor("sumsq", [1, OPC], f32, kind="ExternalOutput")

    with tile.TileContext(nc) as tc:
        with (
            tc.tile_pool(name="const", bufs=1) as cpool,
            tc.tile_pool(name="obj", bufs=6) as opool,
            tc.tile_pool(name="sq", bufs=2) as qpool,
            tc.tile_pool(name="outp", bufs=1) as outpool,
            tc.tile_pool(name="pcos", bufs=2, space="PSUM") as pcos,
            tc.tile_pool(name="pss", bufs=2, space="PSUM") as pss,
        ):
            qn = cpool.tile([128, 2, NQ], f32r)
            nc.sync.dma_start(qn[:, 0, :], qnT_d.ap()[0:128, :])
            nc.sync.dma_start(qn[:, 1, :], qnT_d.ap()[128:256, :])
            ones_t = cpool.tile([128, 128], f32r)
            nc.sync.dma_start(ones_t[:], ones_d.ap())

            rm_all = outpool.tile([128, N_BLK], f32)
            ss_all = outpool.tile([1, OPC], f32)

            off = 0
            for chunk in CHUNKS:
                bpc = chunk // BLK
                ot = opool.tile([128, 2, MAXCHUNK], f32r, tag="ot")
                nc.sync.dma_start(
                    ot[:, 0, 0:chunk], objT_d.ap()[0:128, off : off + chunk]
                )
                nc.sync.dma_start(
                    ot[:, 1, 0:chunk], objT_d.ap()[128:256, off : off + chunk]
                )

                # cosine-score GEMM + per-object max over queries
                b = 0
                while b < bpc:
                    nb = min(RB, bpc - b)
                    ps = pcos.tile([128, RB, NQ], f32, tag="ps")
                    for j in range(nb):
                        bb = b + j
                        nc.tensor.matmul(
                            ps[:, j, :], ot[:, 0, bb * BLK : (bb + 1) * BLK],
                            qn[:, 0, :], start=True, stop=False,
                        )
                        nc.tensor.matmul(
                            ps[:, j, :], ot[:, 1, bb * BLK : (bb + 1) * BLK],
                            qn[:, 1, :], start=False, stop=True,
                        )
                    blk = off // BLK + b
                    nc.vector.reduce_max(
                        rm_all[:, blk : blk + nb], ps[:, 0:nb, :],
                        axis=mybir.AxisListType.X,
                    )
                    b += nb

                # per-object sum of squares: square on ACT, column-sum on PE
                sq = qpool.tile([128, 2, MAXCHUNK], f32r, tag="sq")
                nc.scalar.square(sq[:, :, 0:chunk], ot[:, :, 0:chunk])
                for h in range((chunk + NQ - 1) // NQ):
                    n = min(NQ, chunk - h * NQ)
                    # M=32 ones: only partition 0 of the column-sum result is
                    # consumed, and narrower weights shrink the fp32r
                    # self-load stream on the PE.
                    ssp = pss.tile([128, NQ], f32, tag="ssp")
                    nc.tensor.matmul(
                        ssp[0:32, 0:n], ones_t[:, 0:32],
                        sq[:, 0, h * NQ : h * NQ + n],
                        start=True, stop=False,
                    )
                    nc.tensor.matmul(
                        ssp[0:32, 0:n], ones_t[:, 0:32],
                        sq[:, 1, h * NQ : h * NQ + n],
                        start=False, stop=True,
                    )
                    lo = off + h * NQ
                    nc.scalar.copy(ss_all[0:1, lo : lo + n], ssp[0:1, 0:n])
                off += chunk

            nc.sync.dma_start(rm_d.ap(), rm_all[:])
            nc.sync.dma_start(ss_d.ap(), ss_all[:])

    nc.compile()
    return nc


def _get_nc():
    if "nc" not in _nc_cache:
        _nc_cache["nc"] = _build_nc()
    return _nc_cache["nc"]


def _greedy_nms(scores, thr):
    """Exact numpy mirror of the reference's vectorized greedy NMS."""
    s = scores.copy()
    n = s.shape[0]
    ar = np.arange(n)
    sel = np.zeros(n, dtype=bool)
    while s.size and s.max() != 0.0:
        i = int(s.argmax())
        val = np.float32(thr) * s[i]
        c = (s > val).astype(np.int32)
        left_c = np.where(ar < i, c, 1)
        sup_l = (ar < i) & (np.flip(np.cumprod(np.flip(left_c))) > 0)
        right_c = np.where(ar > i, c, 1)
        sup_r = (ar > i) & (np.cumprod(right_c) > 0)
        s = np.where(sup_l | sup_r | (ar == i), np.float32(0.0), s)
        sel[i] = True
    return sel


def kernel(object_tokens, query_tokens, object_attn_mask, frame_ids,
           query_frame_number, top_k, top_p, nms_threshold):
    global last_exec_time_ns
    import os

    from concourse.bass_utils import run_bass_kernel_spmd

    obj = np.ascontiguousarray(np.asarray(object_tokens)[0], dtype=np.float32)
    qry = np.ascontiguousarray(np.asarray(query_tokens)[0], dtype=np.float32)
    mask = np.asarray(object_attn_mask)[0]
    fids = np.asarray(frame_ids)[0]

    # ---- host marshalling: normalize queries, K-major layout, fp32r grid
    qn = qry / np.clip(np.linalg.norm(qry, axis=1, keepdims=True), 1e-12, None)
    qnT = _round_fp32r(np.ascontiguousarray(qn.T))
    objT = _round_fp32r(np.ascontiguousarray(obj.T))            # [D, N_OBJ]
    ones = np.ones((128, 128), np.float32)

    in_maps = [
        {
            "objT": np.ascontiguousarray(objT[:, c * OPC : (c + 1) * OPC]),
            "qnT": qnT,
            "ones": ones,
        }
        for c in range(N_CORES)
    ]

    nc = _get_nc()
    trace = bool(os.environ.get("BASS_KERNEL_TRACE"))
    try:
        res = run_bass_kernel_spmd(
            nc, in_maps, core_ids=list(range(N_CORES)), trace=trace
        )
    except ModuleNotFoundError:
        # NTFF profiling hook unavailable (axon client without axon_hooks);
        # rerun without tracing.
        res = run_bass_kernel_spmd(nc, in_maps, core_ids=list(range(N_CORES)))
    last_exec_time_ns = res.exec_time_ns

    rowmax = np.concatenate(
        [r["rowmax"].T.reshape(-1) for r in res.results]
    )                                                            # [N_OBJ]
    sumsq = np.concatenate([r["sumsq"].reshape(-1) for r in res.results])

    # ---- scores = rowmax / ||obj|| (device computed both from rounded obj)
    norms = np.sqrt(sumsq, dtype=np.float32)
    cosine_scores = rowmax / np.clip(norms, 1e-12, None).astype(np.float32)
    object_scores = np.where(mask == 1, cosine_scores, np.float32(0.0)).astype(
        np.float32
    )

    # ---- intra-frame NMS: keep per-frame maxima (mirror of reference)
    uniq = np.unique(fids)
    n_frames = int(uniq.max()) + 1
    seg_max = np.full(n_frames, -np.inf, dtype=np.float32)
    np.maximum.at(seg_max, fids, object_scores)
    sel_mask = object_scores >= seg_max[fids]
    sel_idxs = np.nonzero(sel_mask)[0].astype(np.int32)
    sel_scores = object_scores[sel_idxs].copy()

    # ---- query-frame backward suppression of scores >= 0.8
    qfn = int(query_frame_number)
    if bool(np.isin(qfn, uniq)):
        qpos = qfn - int(uniq[0])
        c = (sel_scores[: qpos + 1] >= 0.8).astype(np.int32)
        sup = np.flip(np.cumprod(np.flip(c))) > 0
        sel_scores[: qpos + 1] = np.where(
            sup, np.float32(0.0), sel_scores[: qpos + 1]
        )

    # ---- inter-frame greedy NMS
    if float(nms_threshold) > 0:
        keep = _greedy_nms(sel_scores, np.float32(nms_threshold))
        sel_scores = sel_scores[keep]
        sel_idxs = sel_idxs[keep]

    # ---- top-k (threshold at k-th largest, keep ties)
    k = int(top_k)
    if k < sel_scores.shape[0]:
        thr_k = np.sort(sel_scores)[::-1][k - 1]
        m = sel_scores >= thr_k
        sel_scores = sel_scores[m]
        sel_idxs = sel_idxs[m]

    # ---- top-p (threshold capped by max score)
    thr_p = np.minimum(np.float32(top_p), sel_scores.max())
    m = sel_scores >= thr_p
    sel_scores = sel_scores[m]
    sel_idxs = sel_idxs[m]

    # ---- query argmax, recomputed only for the finally-selected rows
    rows = obj[sel_idxs]
    rows = rows / np.clip(
        np.linalg.norm(rows, axis=1, keepdims=True), 1e-12, None
    )
    query_match_idx = (rows @ qn.T).argmax(axis=1).astype(np.int32)

    return (
        object_scores[None].astype(np.float32),
        sel_scores[None].astype(np.float32),
        sel_idxs[None].astype(np.int32),
        query_match_idx[None],
    )
